# revision 34
# baseline (speedup 1.0000x reference)
"""MoE (8 experts, top-2, d=1024, N=8192) on 8 trn2 NeuronCores.

Strategy (capacity-balanced expert-parallel):
 - Host computes routing (top-2 expert ids per token, fp64 logits for stable
   ordering), chops each expert's token list into 128-token tiles, and
   distributes tiles across cores so that every core gets exactly T tiles.
   Each core holds TWO expert weight slots (primary/secondary); tiles
   0..S-1 use slot 0 and tiles S..T-1 use slot 1 (S is a compile-time
   constant, identical across cores; the experts in the slots are per-core
   DATA). A small covering solver picks (T, S) and the unit assignment;
   if it fails, we fall back to one-expert-per-core with T = max load.
 - Device (per core, SPMD): router logits for its tokens (replicated
   router, bf16), gate g = sigmoid((l_own+br_own) - max_{e!=own}(l_e+br_e))
   via two masked reduce_max (masks are per-tile data), expert matmul
   y = xg @ W[slot] with PSUM K-accumulation, operands bf16, unscaled y
   emitted bf16. Work is grouped G token-tiles per DMA chain; y stores
   issue from the scalar engine, x loads from sync, gates batched to one
   DMA per iteration.
 - Host combines (the weighted all-to-all): out[idx_tile] +=
   g_tile*(y_tile + b[e])  (each token appears in exactly 2 tiles).
"""

import os
from contextlib import ExitStack

import ml_dtypes
import numpy as np

import concourse.bass as bass
import concourse.bacc as bacc
import concourse.mybir as mybir
import concourse.tile as tile
from concourse.bass import ts
from concourse.bass_utils import run_bass_kernel_spmd

N_EXPERTS = 8
TOP_K = 2
D = 1024
N_CORES = 8
P = 128  # partitions
KT = D // P  # number of K tiles (8)
NH = 512  # psum free-dim tile (one bank of fp32)
EW = N_EXPERTS  # logit row width
G = int(os.environ.get("MOE_G", "3"))  # token tiles per group

MM_DTYPE = os.environ.get("MOE_MM_DTYPE", "bf16")  # matmul operand dtype
Y_DTYPE = os.environ.get("MOE_Y_DTYPE", "bf16")  # y output dtype
UNROLL = os.environ.get("MOE_UNROLL", "0") == "1"  # python-unroll repeat (TimelineSim)
# matmul order within a tile:
#   nh: router phase (8 mm) then experts nh-outer (2x8 mm)
#   jr: j-outer, router merged: per j one lhsT load -> yp[0:512], yp[512:], Lp
#   rt: router as wr-stationary matmuls (long token streams, no LS stalls),
#       logits transposed back to token-partition layout via DMA transpose
ORDER = os.environ.get("MOE_ORDER", "nh")
EWP = 32  # padded router width for the rt path (xbar transpose tile rows)
# ablation for HW bottleneck decomposition (bench-only, breaks correctness):
#   noload / nostore / pe (neither) / dmaonly (no compute)
ABLATE = os.environ.get("MOE_ABLATE", "")
# host-side gate multiply: device emits unscaled y; the host combine applies
# g*(y+b) (the "weighted" part of the weighted all-to-all combine). Gates are
# still computed on device and shipped via gout.
HOSTSCALE = os.environ.get("MOE_HOSTSCALE", "1") == "1"
BUFS = int(os.environ.get("MOE_BUFS", "3"))  # x/y pool double-buffering depth
# split the per-tile PSUM accumulator into two 1-bank halves (bufs=6) so the
# drain granularity halves and both DVE+ACT drain every tile
SPLITYP = os.environ.get("MOE_SPLITYP", "1") == "1"
# y-store granularity: group (one DMA per G tiles) or tile (one DMA per tile)
YSTORE = os.environ.get("MOE_YSTORE", "group")
# unroll 2 iterations per hardware-loop body (repeat must be even); lets the
# tile scheduler software-pipeline across the iteration boundary
UNROLL2 = os.environ.get("MOE_UNROLL2", "0") == "1"

LAST_RESULTS = None  # stash of BassKernelResults for test harness inspection

_BUILD_CACHE = {}


def _dt(name):
    return {
        "f32": mybir.dt.float32,
        "f32r": mybir.dt.float32r,
        "bf16": mybir.dt.bfloat16,
        "f16": mybir.dt.float16,
    }[name]


def _npdt(name):
    if name == "bf16":
        return ml_dtypes.bfloat16
    if name == "f16":
        return np.float16
    return np.float32


def _build(T: int, S: int, repeat: int = 1):
    """Build the SPMD Bass module: T token tiles/core, slot boundary S."""
    key = (T, S, MM_DTYPE, Y_DTYPE, repeat, G, UNROLL, ORDER, ABLATE, HOSTSCALE, BUFS, SPLITYP, YSTORE, UNROLL2)
    if key in _BUILD_CACHE:
        return _BUILD_CACHE[key]

    f32 = mybir.dt.float32
    mm_dt = _dt(MM_DTYPE)
    y_dt = _dt(Y_DTYPE)

    ew_in = EWP if ORDER == "rt" else EW
    nc = bacc.Bacc(None, target_bir_lowering=False)
    # inputs (xg_t: tiled tokens [T tile, 128 din-sub, KT, 128 tok] din-major)
    xg_t = nc.declare_dram_parameter("xg_t", [P, T * KT * P], mm_dt, isOutput=False)
    ws = nc.declare_dram_parameter("ws", [2 * D, D], mm_dt, isOutput=False)
    wr = nc.declare_dram_parameter("wr", [D, ew_in], mm_dt, isOutput=False)
    # per-tile mask+bias rows: mbo masks own col, mbw masks all but own col
    mbo = nc.declare_dram_parameter("mbo", [P, T * EW], f32, isOutput=False)
    mbw = nc.declare_dram_parameter("mbw", [P, T * EW], f32, isOutput=False)
    # outputs (y tiled [T, 128 tok, 1024 feat]; gates [T, 128 tok])
    y = nc.declare_dram_parameter("y", [P, T * D], y_dt, isOutput=True)
    gout = nc.declare_dram_parameter("gout", [P, T], f32, isOutput=True)

    with tile.TileContext(nc) as tc, ExitStack() as ctx:
        consts = ctx.enter_context(tc.tile_pool(name="consts", bufs=1))
        xpool = ctx.enter_context(tc.tile_pool(name="x", bufs=BUFS))
        gpool = ctx.enter_context(tc.tile_pool(name="gates", bufs=2))
        gspool = ctx.enter_context(tc.tile_pool(name="gsb", bufs=2))
        ypool = ctx.enter_context(tc.tile_pool(name="y", bufs=BUFS))
        lpsum = ctx.enter_context(
            tc.tile_pool(name="lpsum", bufs=2, space=bass.MemorySpace.PSUM)
        )
        ypsum = ctx.enter_context(
            tc.tile_pool(
                name="ypsum", bufs=6 if SPLITYP else 3, space=bass.MemorySpace.PSUM
            )
        )

        # ---- constants / weights resident in SBUF ----
        w_sb = consts.tile([P, 2, KT, D], mm_dt)
        nc.sync.dma_start(w_sb[:], ws.rearrange("(s kt p) n -> p s kt n", p=P, s=2))

        wr_sb = consts.tile([P, KT, ew_in], mm_dt)
        nc.sync.dma_start(wr_sb[:], wr.rearrange("(kt p) n -> p kt n", p=P))

        mbo_sb = consts.tile([P, T * EW], f32)
        nc.sync.dma_start(mbo_sb[:], mbo[:, :])
        mbw_sb = consts.tile([P, T * EW], f32)
        nc.sync.dma_start(mbw_sb[:], mbw[:, :])

        body_unroll = 2 if (UNROLL2 and repeat > 1 and repeat % 2 == 0) else 1
        rep_cm = None
        if repeat > 1 and not UNROLL:
            rep_cm = tc.For_i(0, repeat // body_unroll, 1)
            rep_cm.__enter__()

        groups = [(g0, min(G, T - g0)) for g0 in range(0, T, G)]
        if repeat > 1 and UNROLL:
            groups = groups * repeat
        elif body_unroll > 1:
            groups = groups * body_unroll

        do_load = ABLATE not in ("noload", "pe", "penr")
        do_store = ABLATE not in ("nostore", "pe", "penr")
        do_pe = ABLATE != "dmaonly"
        do_router = do_pe and ABLATE not in ("norouter", "penr")
        kt_eff = KT // 2 if ABLATE == "halfk" else KT

        xt_static = None
        if not do_load:
            xt_static = consts.tile([P, G * KT * P], mm_dt)
            nc.sync.dma_start(xt_static[:], xg_t[:, : G * KT * P])

        def load_x(g0, gt):
            if not do_load:
                return xt_static
            xt = xpool.tile([P, gt * KT * P], mm_dt, tag="xt")
            nc.sync.dma_start(xt[:], xg_t[:, g0 * KT * P : (g0 + gt) * KT * P])
            return xt

        def router_gates_rt(g0, gt, xt, gsb):
            """Router with wr stationary: 8 long token-stream matmuls into
            [EWP, gt*128] psum, cast+copy to SBUF, DMA-transpose back to
            token-partition layout, then the masked reduce_max gate chain."""
            Lp8 = lpsum.tile([P, gt * P], f32, tag="lp8")
            xv = xt[:].rearrange("p (g k c) -> p g k c", k=KT, c=P)
            for j in range(KT):
                nc.tensor.matmul(
                    Lp8[0:EWP, :],
                    wr_sb[:, j, :],
                    xv[:, :, j, :],
                    start=(j == 0),
                    stop=(j == KT - 1),
                )
            cp = gpool.tile([P, gt * P], mybir.dt.bfloat16, tag="cp")
            nc.vector.tensor_copy(cp[0:EWP, :], Lp8[0:EWP, :])
            Lt = gpool.tile([P, gt * EWP], mybir.dt.bfloat16, tag="lt")
            for tau in range(gt):
                nc.sync.dma_start_transpose(
                    Lt[:, tau * EWP : (tau + 1) * EWP],
                    cp[0:EWP, tau * P : (tau + 1) * P],
                )
            Ltv = Lt[:].rearrange("p (g e) -> p g e", e=EWP)[:, :, 0:EW]
            Lm1 = gpool.tile([P, gt, EW], f32, tag="lm1")
            nc.vector.tensor_add(
                Lm1[:],
                Ltv,
                mbo_sb[:, g0 * EW : (g0 + gt) * EW].rearrange(
                    "p (g e) -> p g e", e=EW
                ),
            )
            Lm2 = gpool.tile([P, gt, EW], f32, tag="lm2")
            nc.vector.tensor_add(
                Lm2[:],
                Ltv,
                mbw_sb[:, g0 * EW : (g0 + gt) * EW].rearrange(
                    "p (g e) -> p g e", e=EW
                ),
            )
            mo = gpool.tile([P, gt, 1], f32, tag="mo")
            nc.vector.reduce_max(mo[:], Lm1[:], axis=mybir.AxisListType.X)
            so = gpool.tile([P, gt, 1], f32, tag="so")
            nc.vector.reduce_max(so[:], Lm2[:], axis=mybir.AxisListType.X)
            nc.vector.tensor_sub(so[:], so[:], mo[:])
            nc.scalar.activation(
                gsb[:, g0 : g0 + gt], so[:, :, 0], mybir.ActivationFunctionType.Sigmoid
            )

        def router_gates(g0, gt, xt, gsb):
            """Router matmuls + gate chain (two masked reduce_max)."""
            if ORDER == "rt":
                return router_gates_rt(g0, gt, xt, gsb)
            Lp = lpsum.tile([P, gt * EW], f32, tag="lp")
            for tau in range(gt):
                for j in range(KT):
                    nc.tensor.matmul(
                        Lp[:, tau * EW : (tau + 1) * EW],
                        xt[:, (tau * KT + j) * P : (tau * KT + j + 1) * P],
                        wr_sb[:, j, :],
                        start=(j == 0),
                        stop=(j == KT - 1),
                    )
            Lm1 = gpool.tile([P, gt * EW], f32, tag="lm1")
            nc.vector.tensor_add(
                Lm1[:], Lp[:], mbo_sb[:, g0 * EW : (g0 + gt) * EW]
            )
            Lm2 = gpool.tile([P, gt * EW], f32, tag="lm2")
            nc.vector.tensor_add(
                Lm2[:], Lp[:], mbw_sb[:, g0 * EW : (g0 + gt) * EW]
            )
            mo = gpool.tile([P, gt, 1], f32, tag="mo")
            nc.vector.reduce_max(
                mo[:],
                Lm1[:].rearrange("p (g e) -> p g e", e=EW),
                axis=mybir.AxisListType.X,
            )
            so = gpool.tile([P, gt, 1], f32, tag="so")
            nc.vector.reduce_max(
                so[:],
                Lm2[:].rearrange("p (g e) -> p g e", e=EW),
                axis=mybir.AxisListType.X,
            )
            nc.vector.tensor_sub(so[:], so[:], mo[:])
            nc.scalar.activation(
                gsb[:, g0 : g0 + gt], so[:, :, 0], mybir.ActivationFunctionType.Sigmoid
            )

        def gate_chain(t, Lp3, gsb):
            """Masked reduce_max gate chain for one tile t; Lp3 is [P, 1, EW]."""
            Lm1 = gpool.tile([P, 1, EW], f32, tag="lm1")
            nc.vector.tensor_add(
                Lm1[:, 0, :], Lp3[:, 0, :], mbo_sb[:, t * EW : (t + 1) * EW]
            )
            Lm2 = gpool.tile([P, 1, EW], f32, tag="lm2")
            nc.vector.tensor_add(
                Lm2[:, 0, :], Lp3[:, 0, :], mbw_sb[:, t * EW : (t + 1) * EW]
            )
            mo = gpool.tile([P, 1, 1], f32, tag="mo")
            nc.vector.reduce_max(mo[:], Lm1[:], axis=mybir.AxisListType.X)
            so = gpool.tile([P, 1, 1], f32, tag="so")
            nc.vector.reduce_max(so[:], Lm2[:], axis=mybir.AxisListType.X)
            nc.vector.tensor_sub(so[:], so[:], mo[:])
            nc.scalar.activation(
                gsb[:, t : t + 1], so[:, 0, :], mybir.ActivationFunctionType.Sigmoid
            )

        def experts(g0, gt, xt, gsb):
            ysb = ypool.tile([P, gt * D], y_dt, tag="ysb")
            for tau in range(gt if do_pe else 0):
                t = g0 + tau
                slot = 0 if t < S else 1
                if SPLITYP:
                    yph = [
                        ypsum.tile([P, NH], f32, tag="yph", name="yph")
                        for _ in range(2)
                    ]
                    for nh in range(2):
                        for j in range(kt_eff):
                            nc.tensor.matmul(
                                yph[nh][:],
                                xt[:, (tau * KT + j) * P : (tau * KT + j + 1) * P],
                                w_sb[:, slot, j, ts(nh, NH)],
                                start=(j == 0),
                                stop=(j == kt_eff - 1),
                            )
                    for nh in range(2):
                        dsth = ysb[:, tau * D + nh * NH : tau * D + (nh + 1) * NH]
                        if nh == 0:
                            nc.vector.tensor_copy(dsth, yph[nh][:])
                        else:
                            nc.scalar.copy(dsth, yph[nh][:])
                    if do_store and YSTORE == "tile":
                        nc.scalar.dma_start(
                            y[:, t * D : (t + 1) * D],
                            ysb[:, tau * D : (tau + 1) * D],
                        )
                    continue
                yp = ypsum.tile([P, D], f32, tag="yp")
                if ORDER == "jr":
                    Lp = lpsum.tile([P, 1, EW], f32, tag="lp")
                    for j in range(KT):
                        lhs = xt[:, (tau * KT + j) * P : (tau * KT + j + 1) * P]
                        for nh in range(D // NH):
                            nc.tensor.matmul(
                                yp[:, ts(nh, NH)],
                                lhs,
                                w_sb[:, slot, j, ts(nh, NH)],
                                start=(j == 0),
                                stop=(j == KT - 1),
                            )
                        nc.tensor.matmul(
                            Lp[:, 0, :],
                            lhs,
                            wr_sb[:, j, :],
                            start=(j == 0),
                            stop=(j == KT - 1),
                        )
                    gate_chain(t, Lp, gsb)
                else:
                    for nh in range(D // NH):
                        for j in range(kt_eff):
                            nc.tensor.matmul(
                                yp[:, ts(nh, NH)],
                                xt[:, (tau * KT + j) * P : (tau * KT + j + 1) * P],
                                w_sb[:, slot, j, ts(nh, NH)],
                                start=(j == 0),
                                stop=(j == kt_eff - 1),
                            )
                dst = ysb[:, tau * D : (tau + 1) * D]
                if HOSTSCALE:
                    if tau % 2 == 0:
                        nc.vector.tensor_copy(dst, yp[:])
                    else:
                        nc.scalar.copy(dst, yp[:])
                else:
                    gsc = gsb[:, t : t + 1]
                    if tau % 2 == 0:
                        nc.vector.tensor_scalar_mul(dst, yp[:], gsc)
                    else:
                        nc.scalar.mul(dst, yp[:], gsc)
            if do_store and not (SPLITYP and YSTORE == "tile"):
                nc.scalar.dma_start(y[:, g0 * D : (g0 + gt) * D], ysb[:])

        # software pipeline: router+gates run one group ahead of experts
        if repeat > 1 and UNROLL:
            n_inner = len(groups) // repeat
        elif body_unroll > 1:
            n_inner = len(groups) // body_unroll
        else:
            n_inner = len(groups)
        gsb_cur = gspool.tile([P, T], f32, tag="gsb", name="gsb")
        if not do_router:
            nc.gpsimd.memset(gsb_cur[:], 1.0)
        xt_cur = load_x(*groups[0])
        if ORDER != "jr" and do_router:
            router_gates(*groups[0], xt_cur, gsb_cur)
        for i, (g0, gt) in enumerate(groups):
            last_of_iter = (i + 1) % n_inner == 0
            if i + 1 < len(groups):
                gsb_nxt = (
                    gspool.tile([P, T], f32, tag="gsb", name="gsb")
                    if last_of_iter
                    else gsb_cur
                )
                if last_of_iter and not do_router:
                    nc.gpsimd.memset(gsb_nxt[:], 1.0)
                xt_nxt = load_x(*groups[i + 1])
                if ORDER != "jr" and do_router:
                    router_gates(*groups[i + 1], xt_nxt, gsb_nxt)
            experts(g0, gt, xt_cur, gsb_cur)
            if last_of_iter and do_pe:
                nc.sync.dma_start(gout[:, :], gsb_cur[:])
            if i + 1 < len(groups):
                xt_cur, gsb_cur = xt_nxt, gsb_nxt

        if rep_cm is not None:
            rep_cm.__exit__(None, None, None)

    nc.compile()
    _BUILD_CACHE[key] = nc
    return nc


def _route(x, Wr, br):
    """Host routing in fp64: per-token top-2 expert ids."""
    n_tokens = x.shape[0]
    logits = x.astype(np.float64) @ Wr.astype(np.float64) + br.astype(np.float64)
    i1 = np.argmax(logits, axis=1)
    l2 = logits.copy()
    l2[np.arange(n_tokens), i1] = -np.inf
    i2 = np.argmax(l2, axis=1)
    return i1, i2


def _solve_assignment(n_tiles_per_e):
    """Pick (T, S) and per-core (primary_e, secondary_e) so that 8 units of
    size S plus 8 units of size T-S cover the per-expert tile demands.
    Exact DP over (S-units used, (T-S)-units used).

    Returns (T, S, pri_experts[8], sec_experts[8]) or None."""
    total = int(sum(n_tiles_per_e))
    t_min = max(1, (total + N_CORES - 1) // N_CORES)
    for T in range(t_min, t_min + 4):
        for S in range((T + 1) // 2, T + 1):
            sz2 = T - S
            # per-expert candidate (p, q) unit counts, Pareto-minimal
            opts = []
            for ne in n_tiles_per_e:
                cand = []
                for p in range(N_CORES + 1):
                    need = ne - S * p
                    q = 0 if need <= 0 else (
                        (need + sz2 - 1) // sz2 if sz2 > 0 else None
                    )
                    if q is None or q > N_CORES:
                        continue
                    if any(p2 <= p and q2 <= q for p2, q2 in cand):
                        continue
                    cand = [(p2, q2) for p2, q2 in cand if not (p <= p2 and q <= q2)]
                    cand.append((p, q))
                if not cand:
                    cand = None
                opts.append(cand)
            if any(o is None for o in opts):
                continue
            # DP: state (pu, qu) -> choice list
            states = {(0, 0): []}
            for cand in opts:
                nxt = {}
                for (pu, qu), hist in states.items():
                    for p, q in cand:
                        k = (pu + p, qu + q)
                        if k[0] <= N_CORES and k[1] <= N_CORES and k not in nxt:
                            nxt[k] = hist + [(p, q)]
                states = nxt
            if not states:
                continue
            (pu, qu), hist = min(states.items(), key=lambda kv: kv[0])
            pri, sec = [], []
            for e, (p, q) in enumerate(hist):
                pri += [e] * p
                sec += [e] * q
            pri += [0] * (N_CORES - len(pri))  # leftover units: pure padding
            sec += [0] * (N_CORES - len(sec))
            return T, S, pri, sec
    return None


def _plan(x, Wr, br, W, b):
    i1, i2 = _route(x, Wr, br)
    idx_per_e = [np.where((i1 == e) | (i2 == e))[0] for e in range(N_EXPERTS)]
    tiles_per_e = [
        [idx[i : i + P] for i in range(0, len(idx), P)] for idx in idx_per_e
    ]
    n_tiles = [len(tl) for tl in tiles_per_e]
    sol = _solve_assignment(n_tiles)
    if sol is None:
        # fallback: one expert per core, T = max tiles
        T = max(max(n_tiles), 1)
        S = T
        pri = list(range(N_CORES))
        sec = list(range(N_CORES))
    else:
        T, S, pri, sec = sol
    # distribute each expert's tile queue over its assigned units
    queues = [list(tl) for tl in tiles_per_e]
    core_tiles = [[] for _ in range(N_CORES)]
    for c in range(N_CORES):
        for e, cnt in ((pri[c], S), (sec[c], T - S)):
            for _ in range(cnt):
                idx = queues[e].pop(0) if queues[e] else np.empty(0, np.int64)
                core_tiles[c].append((e, idx))
    assert all(not q for q in queues), "assignment failed to place all tiles"
    return T, S, core_tiles


def _make_in_maps(x, Wr, br, W, b, T, core_tiles):
    np_mm = _npdt(MM_DTYPE)
    in_maps = []
    for c in range(N_CORES):
        tiles = core_tiles[c]
        xg = np.zeros((T * P, D), dtype=np.float32)
        mbo = np.zeros((T, EW), dtype=np.float32)
        mbw = np.zeros((T, EW), dtype=np.float32)
        experts_used = sorted({e for e, _ in tiles})
        for t, (e, idx) in enumerate(tiles):
            if len(idx):
                xg[t * P : t * P + len(idx)] = x[idx]
            mbo[t] = br
            mbo[t, e] = -1e30
            mbw[t] = -1e30
            mbw[t, e] = br[e]
        # partition-major: xg_t[p, t, j, c] = xg[t*128 + c, j*128 + p]
        xg_t = np.ascontiguousarray(
            xg.reshape(T, P, KT, P).transpose(3, 0, 2, 1).reshape(P, T * KT * P)
        ).astype(np_mm)
        # weight slots: primary = expert of tile 0, secondary = expert of last
        e_pri = tiles[0][0]
        e_sec = tiles[-1][0]
        ws = np.concatenate([W[e_pri], W[e_sec]], axis=0).astype(np_mm)
        wr_arr = Wr
        if ORDER == "rt":
            wr_arr = np.concatenate(
                [Wr, np.zeros((D, EWP - EW), np.float32)], axis=1
            )
        in_maps.append(
            {
                "xg_t": xg_t,
                "ws": np.ascontiguousarray(ws),
                "wr": np.ascontiguousarray(wr_arr).astype(np_mm),
                "mbo": np.broadcast_to(
                    mbo.reshape(1, T * EW), (P, T * EW)
                ).copy(),
                "mbw": np.broadcast_to(
                    mbw.reshape(1, T * EW), (P, T * EW)
                ).copy(),
            }
        )
    return in_maps


def _prep(inputs):
    x = np.asarray(inputs["x"], dtype=np.float32)
    Wr = np.asarray(inputs["Wr"], dtype=np.float32)
    br = np.asarray(inputs["br"], dtype=np.float32)
    W = np.asarray(inputs["W"], dtype=np.float32)
    b = np.asarray(inputs["b"], dtype=np.float32)
    T, S, core_tiles = _plan(x, Wr, br, W, b)
    # sanity: slot boundary must match data layout (tile t uses slot 0 iff t<S)
    for c in range(N_CORES):
        tiles = core_tiles[c]
        for t, (e, _) in enumerate(tiles):
            want = tiles[0][0] if t < S else tiles[-1][0]
            assert e == want, (c, t, e, want)
    in_maps = _make_in_maps(x, Wr, br, W, b, T, core_tiles)
    plan = {
        "T": T,
        "S": S,
        "core_tiles": core_tiles,
        "n_tokens": x.shape[0],
        "b": b,
    }
    return in_maps, plan


def kernel(**inputs) -> np.ndarray:
    global LAST_RESULTS
    in_maps, plan = _prep(inputs)
    T, S = plan["T"], plan["S"]
    nc = _build(T, S)
    res = run_bass_kernel_spmd(nc, in_maps, core_ids=list(range(N_CORES)))
    LAST_RESULTS = res

    n_tokens, b = plan["n_tokens"], plan["b"]
    out = np.zeros((n_tokens, D), dtype=np.float32)
    for c in range(N_CORES):
        # y [P, T*D]: y[p, t*D + f] = tile t, token-slot p, feature f
        ye = (
            res.results[c]["y"]
            .reshape(P, T, D)
            .transpose(1, 0, 2)
            .astype(np.float32)
        )
        ge = res.results[c]["gout"].T.reshape(T, P)
        for t, (e, idx) in enumerate(plan["core_tiles"][c]):
            n = len(idx)
            if n == 0:
                continue
            if HOSTSCALE:
                out[idx] += ge[t, :n, None] * (ye[t, :n] + b[e][None, :])
            else:
                out[idx] += ye[t, :n] + ge[t, :n, None] * b[e][None, :]
    return out
